# revision 1
# baseline (speedup 1.0000x reference)
"""GATv2 layer on 8 Trainium2 NeuronCores.

Problem (hardcoded): B=4, N=256, D=256, HEADS=8, DH=32, neg_slope=0.2.

    X = (H @ W_lin) split into heads               [B, h, N, 32]
    e = leaky_relu(Xi + Xj, 0.2) . a[h]            [B, h, N, N]
    e += ln(A0 + 1e-8);  e = -inf outside mask
    attn = softmax_j(e);  Y = attn @ X  (heads merged) @ W_out

Sharding: 8 cores = (batch b = core//2) x (head-group g = core%2, 4 heads
each).  Every core computes a full [N, D] partial of Y[b] (its 4 heads'
contribution through W_out rows g*128:(g+1)*128); host sums the two
partials per batch.  SPMD: all cores run the same program on pre-sliced
inputs (no partition-id branching).

Math trick: leaky(x) = 0.2*x + 0.8*relu(x), so with q = 0.2 * a^T X:

    e[h,i,j] = 0.8 * sum_d a[h,d]*relu(X[h,d,i]+X[h,d,j]) + q[h,i] + q[h,j]

The pairwise relu pass packs all 4 local heads' dims on the 128 SBUF
partitions (Xt[(h,d), i]) and is one fused op per (query, j-half),
split DVE/ACT ~22/10 by the per-c engine table (GpSimd measured 3us/op
and its shared-port locks stall concurrent DVE ops - never use).  The
d-reduction is a PE matmul with a sliding-window view of a zero-padded
block-diagonal 0.8*a weight matrix, accumulating rows 4c+h for 32
query nodes c into one [128, 512] PSUM tile.

Precision: X / relu scores / AV inputs are fp16 (0.05% rounding);
logits (e + mask bias) stay fp32; exp output (attention weights) is
bf16 for range safety.  PSUM accumulation is always fp32.

The fill PSUM drain is regrouped from (head,c)-packed partitions to
query-major e tiles by 8 column-tiled PE permutation matmuls per
group (lhsT = identity slice) instead of 32 serialized SP-queue DMAs.
"""

import numpy as np

try:
    import concourse.bass as bass
except ImportError:  # pragma: no cover - fallback for bare containers
    import sys

    sys.path.insert(0, "/opt/trn_rl_repo")
    import concourse.bass as bass

import concourse.mybir as mybir
import concourse.tile as tile
from concourse import masks
from concourse.bass_utils import run_bass_kernel_spmd

F32 = mybir.dt.float32
F16 = mybir.dt.float16
BF16 = mybir.dt.bfloat16
U8 = mybir.dt.uint8
AF = mybir.ActivationFunctionType
ALU = mybir.AluOpType

N = 256
D = 256
HEADS = 8
DH = 32
HL = 4  # heads per core
P = 128
NCORES = 8

# Per-c engine assignment for the pairwise relu pass.  Approx per-c cost:
# DVE ~1.7us, ACT (1x LUT rate, dtype-independent) ~3.6us.  GpSimd
# measured 3.1us/op AND its shared-port locks stall concurrent DVE ops
# 4x (measured 159us kernel vs 75us without) - never use it here.
_ACT_C = {0, 3, 6, 10, 13, 16, 20, 23, 26, 29}
_GPS_C = set()


def _gen_engine(c):
    if c in _ACT_C:
        return "act"
    if c in _GPS_C:
        return "gps"
    return "dve"


def _split_multiwait(nc, maxw=1):
    """Walrus codegen here rejects instructions with >1 sem wait ("Too many
    sync wait commands", CoreV3GenImpl setupSyncWait).  Tile's kernel-tail
    drain carries one wait per ticked processor; hoist the extras into
    single-wait NoOps on the same engine just before the instruction."""
    import bass_rust

    n = 0
    for f in nc.m.functions:
        for b in f.blocks:
            new, changed = [], False
            for i in b.instructions:
                si = i.sync_info
                ow = list(si.on_wait) if (si is not None and si.on_wait) else []
                if len(ow) > maxw:
                    extra, keep = ow[:-maxw], ow[-maxw:]
                    for w in extra:
                        nop = mybir.InstNoOp(name=f"I-waitsplit-{n}")
                        n += 1
                        nop.engine = i.engine
                        nop.sync_info = bass_rust.SyncInfo(on_wait=[w], on_update=[])
                        new.append(nop)
                    i.sync_info = bass_rust.SyncInfo(
                        on_wait=keep,
                        on_update=list(si.on_update) if si.on_update else [],
                    )
                    changed = True
                new.append(i)
            if changed:
                b.instructions = new


def build_module():
    nc = bass.Bass("TRN2", target_bir_lowering=False, debug=False)

    hb = nc.dram_tensor("Hb", [N, D], F32, kind="ExternalInput").ap()
    wlg = nc.dram_tensor("WlinG", [D, P], F32, kind="ExternalInput").ap()
    wog = nc.dram_tensor("WoutG", [P, D], F32, kind="ExternalInput").ap()
    ag = nc.dram_tensor("aG", [HL, DH], F32, kind="ExternalInput").ap()
    mask_d = nc.dram_tensor("mask", [N, N], U8, kind="ExternalInput").ap()
    a0_d = nc.dram_tensor("A0", [N, N], F32, kind="ExternalInput").ap()
    out_d = nc.dram_tensor("out", [N, D], F32, kind="ExternalOutput").ap()

    with tile.TileContext(nc) as tc:
        _body(nc, tc, hb, wlg, wog, ag, mask_d, a0_d, out_d)
    return nc


def _body(nc, tc, hb, wlg, wog, ag, mask_d, a0_d, out_d):
    from contextlib import ExitStack

    ctx = ExitStack()
    with ctx:
        const = ctx.enter_context(tc.tile_pool(name="const", bufs=1))
        work = ctx.enter_context(tc.tile_pool(name="work", bufs=3))
        spool = ctx.enter_context(tc.tile_pool(name="spool", bufs=12))
        drpool = ctx.enter_context(tc.tile_pool(name="drpool", bufs=3))
        ps = ctx.enter_context(tc.tile_pool(name="ps", bufs=4, space="PSUM"))
        fillps = ctx.enter_context(tc.tile_pool(name="fillps", bufs=2, space="PSUM"))
        epsp = ctx.enter_context(tc.tile_pool(name="epsp", bufs=1, space="PSUM"))

        # ---------------- setup: loads -------------------------------
        ident = const.tile([P, P], F32, name="ident", tag="ident")
        masks.make_identity(nc, ident[:])
        identb = const.tile([P, P], BF16, name="identb", tag="identb")
        nc.vector.tensor_copy(identb[:], ident[:])
        identh = const.tile([P, P], F16, name="identh", tag="identh")
        nc.vector.tensor_copy(identh[:], ident[:])

        # X-pipeline inputs first (they gate the PE chain), mask/A0 after;
        # small/odd loads go on the ACT HWDGE ring in parallel.
        hbt = [const.tile([P, D], F32, name=f"hbt{k}", tag=f"hbt{k}") for k in range(2)]
        for k in range(2):
            nc.sync.dma_start(out=hbt[k][:], in_=hb[k * P : (k + 1) * P, :])
        wlt = [const.tile([P, P], F32, name=f"wlt{k}", tag=f"wlt{k}") for k in range(2)]
        for k in range(2):
            nc.sync.dma_start(out=wlt[k][:], in_=wlg[k * P : (k + 1) * P, :])
        mskt = [const.tile([P, N], U8, name=f"mskt{k}", tag=f"mskt{k}") for k in range(2)]
        a0t = [const.tile([P, N], F32, name=f"a0t{k}", tag=f"a0t{k}") for k in range(2)]
        for k in range(2):
            nc.sync.dma_start(out=mskt[k][:], in_=mask_d[k * P : (k + 1) * P, :])
            nc.sync.dma_start(out=a0t[k][:], in_=a0_d[k * P : (k + 1) * P, :])

        ablk = const.tile([P, HL], F32, name="ablk", tag="ablk")
        nc.gpsimd.memset(ablk[:], 0.0)
        for h in range(HL):
            nc.scalar.dma_start(
                out=ablk[h * DH : (h + 1) * DH, h : h + 1],
                in_=ag[h : h + 1, :],
            )
        wot = const.tile([P, D], F32, name="wot", tag="wot")
        nc.scalar.dma_start(out=wot[:], in_=wog[:, :])

        wotb = const.tile([P, D], F16, name="wotb", tag="wotb")
        nc.vector.tensor_copy(wotb[:], wot[:])

        ones_t = const.tile([1, P], F32, name="ones_t", tag="ones_t")
        nc.gpsimd.memset(ones_t[:], 1.0)
        eps_col = const.tile([P, 1], F32, name="eps_col", tag="eps_col")
        nc.gpsimd.memset(eps_col[:], 1e-8)

        # ---------------- HT = Hb^T, Xp = Hb @ WlinG (fp32), Xt fp16 -
        # HAM warmup matmuls are interleaved into the X-prep chain's
        # dependency gaps (PE queue is in-order, so a block of warms ahead
        # of the chain would delay it); they keep the PE activity window
        # busy so fills start at 2.4 GHz.  Kept live via warmz -> ablkh.
        wrm = fillps.tile([P, 2 * N], F32, name="wrm", tag="fill")

        def _warm(n, use_b=True):
            for _ in range(n):
                if use_b:
                    nc.tensor.matmul(
                        wrm[:, :P], lhsT=identb[:], rhs=identb[:],
                        start=True, stop=True,
                    )
                else:
                    nc.tensor.matmul(
                        wrm[:, :P], lhsT=ident[:], rhs=ident[:],
                        start=True, stop=True,
                    )

        _warm(1, use_b=False)
        ht = [const.tile([P, N], F32, name=f"ht{k}", tag=f"ht{k}") for k in range(2)]
        for cb in range(2):  # column block of Hb = partition block of HT
            for ib in range(2):
                tp = ps.tile([P, N], F32, name="ps_t", tag="ps_t")
                nc.tensor.transpose(
                    tp[:, :P], hbt[ib][:, cb * P : (cb + 1) * P], ident[:]
                )
                nc.vector.tensor_copy(ht[cb][:, ib * P : (ib + 1) * P], tp[:, :P])
            _warm(2)

        xpb = [const.tile([P, P], F16, name=f"xpb{ib}", tag=f"xpb{ib}") for ib in range(2)]
        xpbb = [const.tile([P, P], BF16, name=f"xpbb{ib}", tag=f"xpbb{ib}") for ib in range(2)]
        for ib in range(2):
            xps = ps.tile([P, N], F32, name="ps_t", tag="ps_t")
            for k in range(2):
                nc.tensor.matmul(
                    xps[:, :P],
                    lhsT=ht[k][:, ib * P : (ib + 1) * P],
                    rhs=wlt[k][:],
                    start=(k == 0),
                    stop=(k == 1),
                )
            nc.scalar.copy(xpb[ib][:], xps[:, :P])
            nc.vector.tensor_copy(xpbb[ib][:], xps[:, :P])
            _warm(2)

        xtb = const.tile([P, N], F16, name="xtb", tag="xtb")
        for ib in range(2):
            tph = ps.tile([P, N], F16, name="ps_t", tag="ps_t")
            nc.tensor.transpose(tph[:, :P], xpb[ib][:], identh[:])
            nc.vector.tensor_copy(xtb[:, ib * P : (ib + 1) * P], tph[:, :P])
            _warm(2)

        # fp32 image of X^T: per-partition scalar operands (DVE scalar1 /
        # ACT bias) must be fp32; values identical to the fp16 xtb.
        xtf = const.tile([P, N], F32, name="xtf", tag="xtf")
        nc.vector.tensor_copy(xtf[:], xtb[:])

        # Zbig: [128, 192] zeros with 0.8*aG[h] block at rows h*32, col
        # 32+32h; window Zbig[:, 32-c:160-c] as lhsT puts head h's query-c
        # reduction at out partition h*32+c.  Built on GpSimd (off the DVE
        # queue); warmz keeps the warmup matmuls live.
        warmz = const.tile([P, HL], F32, name="warmz", tag="warmz")
        nc.vector.tensor_scalar(
            out=warmz[:], in0=wrm[:, :HL], scalar1=0.0, scalar2=None, op0=ALU.mult
        )
        ablkh = const.tile([P, HL], F16, name="ablkh", tag="ablkh")
        nc.gpsimd.tensor_tensor(out=ablkh[:], in0=ablk[:], in1=warmz[:], op=ALU.add)
        zt = const.tile([P, 192], F16, name="zt", tag="zt")
        nc.gpsimd.memset(zt[:], 0.0)
        nc.gpsimd.tensor_scalar(
            out=zt[:, DH : DH + HL * DH : DH],
            in0=ablkh[:],
            scalar1=0.8,
            scalar2=None,
            op0=ALU.mult,
        )

        # ---------------- q = 0.2 * a^T X  --------------------------
        qps = ps.tile([HL, N], F32, name="ps_q", tag="ps_t")
        nc.tensor.matmul(qps[:], lhsT=ablkh[:], rhs=xtb[:], start=True, stop=True)
        q_sb = const.tile([HL, N], F32, name="q_sb", tag="q_sb")
        nc.scalar.activation(q_sb[:], qps[:], AF.Copy, bias=0.0, scale=0.2)

        # q broadcast along partitions (q_j along free), per head
        qrow = [const.tile([1, N], F32, name=f"qrow{h}", tag=f"qrow{h}") for h in range(HL)]
        for h in range(HL):
            nc.sync.dma_start(out=qrow[h][:], in_=q_sb[h : h + 1, :])
        qb = [const.tile([P, N], F32, name=f"qb{h}", tag=f"qb{h}") for h in range(HL)]
        for h in range(HL):
            qbs = ps.tile([P, N], F32, name="ps_t", tag="ps_t")
            nc.tensor.matmul(
                qbs[:], lhsT=ones_t[:], rhs=qrow[h][:], start=True, stop=True
            )
            if h % 2 == 0:
                nc.scalar.copy(qb[h][:], qbs[:])
            else:
                nc.vector.tensor_copy(qb[h][:], qbs[:])

        mtile = [const.tile([P, N], F16, name=f"mtile{it}", tag=f"mtile{it}") for it in range(2)]
        mq = [
            [const.tile([P, N], F16, name=f"mq{h}_{it}", tag=f"mq{h}_{it}") for it in range(2)]
            for h in range(HL)
        ]
        for it in range(2):
            nc.gpsimd.memset(mtile[it][:], -60000.0)

        # ------- pairwise relu pass + PE reduce + per-half tail ------
        # Two phases (query halves it=0,1): fills 2it,2it+1 then that
        # half's softmax/AV/projection, so the second half's relu pass
        # overlaps the first half's tail work.
        e_raw0r = [
            const.tile([P, P], F16, name=f"e_raw0r_{h}", tag=f"e_raw0r_{h}")
            for h in range(HL)
        ]
        ptc0 = const.tile([P, HL * N], BF16, name="ptc0", tag="ptc0")
        # phase-1 softmax tiles packed in head PAIRS so two wide exps
        # replace four serialized exp+accum ops on the tail critical path
        pt1 = [const.tile([P, 2 * N], BF16, name=f"pt1_{p}", tag=f"pt1_{p}") for p in range(2)]
        es1 = [const.tile([P, 2 * N], F32, name=f"es1_{p}", tag=f"es1_{p}") for p in range(2)]
        rec = [
            [const.tile([P, 1], F32, name=f"rec{h}_{it}", tag=f"rec{h}_{it}") for it in range(2)]
            for h in range(HL)
        ]
        att = [
            [const.tile([P, N], BF16, name=f"att{h}_{jh}", tag=f"att{h}_{jh}") for jh in range(2)]
            for h in range(HL)
        ]
        ytile = [const.tile([P, P], F16, name=f"ytile{ib}", tag=f"ytile{ib}") for ib in range(2)]
        yt = const.tile([P, N], F16, name="yt", tag="yt")

        for it in range(2):
            # Phase it=1 generates only the j>=128 half: the (i>=128, j<128)
            # quadrant of the symmetric relu-score equals the transpose of
            # phase 0's (i<128, j>=128) quadrant (PE-transposed below).
            jw = N if it == 0 else P
            j0 = N - jw
            epsall = epsp.tile([P, HL * jw], F32, name="epsall", tag="eps")
            eps = [epsall[:, h * jw : (h + 1) * jw] for h in range(HL)]
            if it == 1:
                # (i>=128, j<128) quadrant = transpose of phase 0's raw
                # (i<128, j>=128); emitted before this phase's fills so it
                # runs off the tail's critical path
                for h in range(HL):
                    tpe = ps.tile([P, P], F16, name="ps_t", tag="ps_t")
                    nc.tensor.transpose(tpe[:], e_raw0r[h][:], identh[:])
                    nc.vector.tensor_tensor(
                        out=es1[h // 2][:, (h % 2) * N : (h % 2) * N + P],
                        in0=tpe[:],
                        in1=mq[h][1][:, 0:P],
                        op=ALU.add,
                    )
            for G in (2 * it, 2 * it + 1):
                fps = fillps.tile([P, 2 * jw], F32, name="fill", tag="fill")
                for c in range(32):
                    st = spool.tile([P, 2 * jw], F16, name="st", tag="st")
                    for half in range(2):
                        i = 64 * G + 32 * half + c
                        dst = st[:, half * jw : (half + 1) * jw]
                        eng = _gen_engine(c)
                        if eng == "act":
                            nc.scalar.activation(
                                dst, xtb[:, j0:N], AF.Relu, bias=xtf[:, i : i + 1]
                            )
                        elif eng == "gps":
                            nc.gpsimd.tensor_scalar(
                                out=dst,
                                in0=xtb[:, j0:N],
                                scalar1=xtf[:, i : i + 1],
                                scalar2=0.0,
                                op0=ALU.add,
                                op1=ALU.max,
                            )
                        else:
                            nc.vector.tensor_scalar(
                                out=dst,
                                in0=xtb[:, j0:N],
                                scalar1=xtf[:, i : i + 1],
                                scalar2=0.0,
                                op0=ALU.add,
                                op1=ALU.max,
                            )
                    nc.tensor.matmul(
                        fps[:],
                        lhsT=zt[:, DH - c : 160 - c],
                        rhs=st[:],
                        start=(c == 0),
                        stop=(c == 31),
                    )
                dr = drpool.tile([P, 2 * jw], F16, name="dr", tag="dr")
                nc.scalar.copy(dr[:], fps[:])
                # regroup (h,c)-packed rows into query-major e tiles with
                # column-tiled PE permutation matmuls (lhsT = ident slice)
                for h in range(HL):
                    for half in range(2):
                        r0 = (64 * G + 32 * half) % P
                        nc.tensor.matmul(
                            epsall[r0 : r0 + 32, h * jw : (h + 1) * jw],
                            lhsT=identh[:, h * DH : (h + 1) * DH],
                            rhs=dr[:, half * jw : (half + 1) * jw],
                            start=True,
                            stop=True,
                            tile_position=(0, r0),
                        )

            if it == 0:
                # mask bias built here (emitted after the fills) so it does
                # not block the relu ops at the head of the DVE queue
                for it2 in range(2):
                    lna = work.tile([P, N], F16, name="lna", tag="lna")
                    nc.scalar.activation(lna[:], a0t[it2][:], AF.Ln, bias=eps_col[:])
                    nc.vector.copy_predicated(mtile[it2][:], mskt[it2][:], lna[:])
                for h in range(HL):
                    for it2 in range(2):
                        nc.vector.tensor_tensor(
                            out=mq[h][it2][:],
                            in0=mtile[it2][:],
                            in1=qb[h][:],
                            op=ALU.add,
                        )
                # packed softmax: one exp over all 4 heads (the q_i bias
                # is dropped - it cancels in the softmax ratio); per-head
                # rowsums on DVE
                e3c = const.tile([P, HL * N], F32, name="e3c", tag="e3c")
                for h in range(HL):
                    nc.vector.tensor_tensor(
                        out=e3c[:, h * N : (h + 1) * N],
                        in0=eps[h],
                        in1=mq[h][0][:],
                        op=ALU.add,
                    )
                    # raw right half for phase 1's transposed quadrant
                    nc.scalar.copy(e_raw0r[h][:], epsall[:, h * jw + P : h * jw + N])
                nc.scalar.activation(ptc0[:], e3c[:], AF.Exp)
                for h in range(HL):
                    den = work.tile([P, 1], F32, name="den", tag="den")
                    nc.vector.tensor_reduce(
                        den[:],
                        ptc0[:, h * N : (h + 1) * N],
                        axis=mybir.AxisListType.X,
                        op=ALU.add,
                    )
                    nc.vector.reciprocal(rec[h][0][:], den[:])
            else:
                for p in range(2):
                    for h in (2 * p, 2 * p + 1):
                        o = (h % 2) * N
                        nc.vector.tensor_tensor(
                            out=es1[p][:, o + P : o + N],
                            in0=eps[h],
                            in1=mq[h][1][:, P:N],
                            op=ALU.add,
                        )
                    nc.scalar.activation(pt1[p][:], es1[p][:], AF.Exp)
                    for h in (2 * p, 2 * p + 1):
                        o = (h % 2) * N
                        den = work.tile([P, 1], F32, name="den", tag="den")
                        nc.vector.tensor_reduce(
                            den[:],
                            pt1[p][:, o : o + N],
                            axis=mybir.AxisListType.X,
                            op=ALU.add,
                        )
                        nc.vector.reciprocal(rec[h][1][:], den[:])

            # attn^T via PE for this half
            for h in range(HL):
                for jh in range(2):
                    tpb = ps.tile([P, N], BF16, name="ps_t", tag="ps_t")
                    src_pt = (
                        ptc0[:, h * N + jh * P : h * N + (jh + 1) * P]
                        if it == 0
                        else pt1[h // 2][
                            :, (h % 2) * N + jh * P : (h % 2) * N + (jh + 1) * P
                        ]
                    )
                    nc.tensor.transpose(tpb[:, :P], src_pt, identb[:])
                    if it == 0:
                        nc.scalar.copy(att[h][jh][:, it * P : (it + 1) * P], tpb[:, :P])
                    else:
                        nc.vector.tensor_copy(
                            att[h][jh][:, it * P : (it + 1) * P], tpb[:, :P]
                        )

            # AV + 1/den scale for i-block it
            ib = it
            for h in range(HL):
                yps = ps.tile([P, DH], F32, name="ps_y", tag="ps_t")
                for k in range(2):
                    nc.tensor.matmul(
                        yps[:],
                        lhsT=att[h][k][:, ib * P : (ib + 1) * P],
                        rhs=xpbb[k][:, h * DH : (h + 1) * DH],
                        start=(k == 0),
                        stop=(k == 1),
                    )
                nc.vector.tensor_scalar(
                    out=ytile[ib][:, h * DH : (h + 1) * DH],
                    in0=yps[:],
                    scalar1=rec[h][ib][:],
                    scalar2=None,
                    op0=ALU.mult,
                )

            # out rows for this i-block: transpose Y then @ WoutG
            tph = ps.tile([P, N], F16, name="ps_t", tag="ps_t")
            nc.tensor.transpose(tph[:, :P], ytile[ib][:], identh[:])
            nc.scalar.copy(yt[:, ib * P : (ib + 1) * P], tph[:, :P])
            ops_ = ps.tile([P, N], F32, name="ps_t", tag="ps_t")
            nc.tensor.matmul(
                ops_[:],
                lhsT=yt[:, ib * P : (ib + 1) * P],
                rhs=wotb[:],
                start=True,
                stop=True,
            )
            osb = work.tile([P, N], F32, name="osb", tag="osb")
            nc.scalar.copy(osb[:], ops_[:])
            nc.sync.dma_start(out=out_d[ib * P : (ib + 1) * P, :], in_=osb[:])


_NC_CACHE = None


def _get_module():
    global _NC_CACHE
    if _NC_CACHE is None:
        nc = build_module()
        _split_multiwait(nc)  # HW-compile only; breaks CoreSim bookkeeping
        _NC_CACHE = nc
    return _NC_CACHE


def make_in_maps(H, mask, A0, W_lin, a, W_out):
    H = np.ascontiguousarray(np.asarray(H, dtype=np.float32))
    W_lin = np.ascontiguousarray(np.asarray(W_lin, dtype=np.float32))
    W_out = np.ascontiguousarray(np.asarray(W_out, dtype=np.float32))
    a = np.ascontiguousarray(np.asarray(a, dtype=np.float32))
    A0 = np.ascontiguousarray(np.asarray(A0, dtype=np.float32))
    mask_u8 = np.ascontiguousarray(np.asarray(mask).astype(np.uint8))
    in_maps = []
    for c in range(NCORES):
        b, g = divmod(c, 2)
        in_maps.append(
            {
                "Hb": H[b],
                "WlinG": np.ascontiguousarray(W_lin[:, g * P : (g + 1) * P]),
                "WoutG": np.ascontiguousarray(W_out[g * P : (g + 1) * P, :]),
                "aG": np.ascontiguousarray(a[g * HL : (g + 1) * HL, :]),
                "mask": mask_u8,
                "A0": A0,
            }
        )
    return in_maps


def run_raw(H, mask, A0, W_lin, a, W_out, **kw):
    nc = _get_module()
    in_maps = make_in_maps(H, mask, A0, W_lin, a, W_out)
    return run_bass_kernel_spmd(nc, in_maps, list(range(NCORES)), **kw)


def assemble(results):
    parts = [results[c]["out"] for c in range(NCORES)]
    out = np.stack(
        [parts[2 * b].astype(np.float32) + parts[2 * b + 1] for b in range(4)]
    )
    return out.astype(np.float32)


def kernel(H, mask, A0, W_lin, a, W_out):
    res = run_raw(H, mask, A0, W_lin, a, W_out)
    return assemble(res.results)

